# revision 16
# baseline (speedup 1.0000x reference)
"""Paged-attention decode kernel for Trainium2, sharded over 8 NeuronCores by KV head.

Problem (hardcoded): B=16, 32 Q heads / 8 KV heads (GQA n_rep=4), D=128,
paged f32 KV cache of 4096 blocks x 16 tokens, 256-entry block tables,
context_lens in [1, 4096), one new (k, v) token per sequence scattered at
slot_mapping before attention.

Strategy per core (1 KV head + its 4 Q heads, all 16 batches):
  host: per-head fused KV shard in bf16 block-row layout [4096, 4112]:
  row b = [K block b (16*128) | V block b (16*129)], where each V token's
  128 values are followed by a 1.0 (so the PV matmul accumulates the softmax
  denominator free). The one-token scatter is applied on host.

  Batches are permuted into two "slabs" of 8 with balanced total block
  counts; each group of <=128 gathered block rows lives entirely in one slab,
  so its 32 query-head columns (8 batches x 4 heads) map to a 32-aligned
  PSUM partition slab (matmul output base must be 0/32/64/96). The host
  unpermutes batches when reassembling the output.

  device, per group g (all bf16 on PE => 1 cycle/row + fast weight load):
    ONE indirect DMA gathers the group's fused KV block rows (8.2KB per
    partition; padded slots bounds-skipped).
    PE emits all 16 transposes xkv[:, t*128:(t+1)*128] -> ktp [d, blk]
    (PSUM bf16) first; DVE/ACT alternate copying them to SBUF while PE
    streams the remaining transposes, so the 16 scores matmuls (lhsT=kt,
    rhs=qt[d, 32 slab cols] -> [blk, 32] PSUM) never stall. One ACT exp
    (scale folded) writes the group's e tile [blk, (t=16, 32)] IN FULL; one
    DVE mask-mult zeroes invalid tokens / padded blocks / non-owning
    batches. For t: matmul(out=acc[slab*32:+32, 0:129], lhsT=e[:, t*32:
    (t+1)*32], rhs=xkv[:, VOFF + t*129 :+129]) accumulates out + den.
    No SBUF memsets in the prologue: e/sc tiles are fully rewritten every
    pass, and stale xkv tail rows are finite data zeroed by the mask (only
    a buffer whose FIRST writer is a partial group gets a one-time memset
    of its tail rows).
  final: out = acc[:, :128] * reciprocal(acc[:, 128]); DMA out (rows in
  permuted batch-major order; host unpermutes).

The Bass program is JIT-specialized on the group structure (a small tuple
derived from context_lens); compiled programs are cached per structure.
"""

import numpy as np
import ml_dtypes

import concourse.bass as bass
import concourse.bacc as bacc
import concourse.mybir as mybir
import concourse.tile as tile
from concourse.bass_utils import run_bass_kernel_spmd
from concourse.masks import make_identity

F32 = mybir.dt.float32
BF16 = mybir.dt.bfloat16
F8E3 = mybir.dt.float8e3
U8 = mybir.dt.uint8
I32 = mybir.dt.int32

B = 16
NUM_HEADS = 32
KV = 8
D = 128
SCALE = 0.08838834764831845
NUM_BLOCKS = 4096
BLOCK_SIZE = 16
BLOCKS_PER_SEQ = 256
N_REP = NUM_HEADS // KV   # 4
BH = B * N_REP            # 64
GRP = 128                 # blocks per group (one per partition)
SLABB = 8                 # batches per slab (8*4 heads = 32 partitions)
SW = SLABB * N_REP        # 32: query columns per slab
KROW = BLOCK_SIZE * D           # 2048 B: K part of a row (fp8 e3m4)
VTOK = D + 1                    # 129: 128 v + 1.0 (bf16)
VTOKB = VTOK * 2                # 258 B per V token
VROW = BLOCK_SIZE * VTOKB       # 4128 B: V part of a row
ROW = KROW + VROW               # 6176 B fused row (uint8-typed)
VOFF = KROW                     # V byte offset within a row
ECOL = BLOCK_SIZE * SW          # 512 cols per e tile
OOB = 8192                      # block-table marker for padded slots
PF = 7                          # gather prefetch distance (groups)

_kernel_cache: dict = {}


def plan_batches(context_lens):
    """Permute batches into two 8-batch slabs with balanced block counts."""
    nb = np.maximum(1, -(-np.asarray(context_lens).astype(np.int64) // BLOCK_SIZE))
    nb = np.minimum(nb, BLOCKS_PER_SEQ)
    order = np.argsort(-nb, kind="stable")
    sets = ([], [])
    sums = [0, 0]
    for b in order:
        i = 0 if (sums[0] <= sums[1] and len(sets[0]) < SLABB) or \
                 len(sets[1]) >= SLABB else 1
        sets[i].append(int(b))
        sums[i] += int(nb[b])
    perm = np.array(sorted(sets[0]) + sorted(sets[1]), dtype=np.int64)
    return perm, nb


def plan_groups(context_lens):
    perm, nb = plan_batches(context_lens)
    groups = []       # (slab, start_in_slab, cnt)
    for slab in range(2):
        tot = int(sum(nb[perm[slab * SLABB + j]] for j in range(SLABB)))
        i = 0
        while i < tot:
            c = min(GRP, tot - i)
            groups.append((slab, i, c))
            i += c
    return groups, perm, nb


def build_program_v(key: tuple, rep: int = 1, variant: str = "full"):
    """key = tuple of (slab, cnt) per group."""
    G = len(key)
    mcols = ECOL * G

    nc = bacc.Bacc("TRN2", target_bir_lowering=False, debug=False)
    kvsh = nc.dram_tensor("kvsh", [NUM_BLOCKS, ROW], U8, kind="ExternalInput")
    qt = nc.dram_tensor("qt", [D, BH], BF16, kind="ExternalInput")
    btt = nc.dram_tensor("btt", [GRP, G], I32, kind="ExternalInput")
    msk = nc.dram_tensor("msk", [GRP, mcols], F8E3, kind="ExternalInput")
    out = nc.dram_tensor("out", [BH, D], F32, kind="ExternalOutput")

    do_gather = variant in ("full", "dma", "overlap")
    do_comp = variant in ("full", "compute", "notrans", "overlap")
    do_trans = variant in ("full", "compute", "overlap")
    use_const = variant == "overlap"

    with tile.TileContext(nc) as tc:
        with (
            tc.tile_pool(name="const", bufs=1) as cpool,
            tc.tile_pool(name="xkv", bufs=1) as xkvpool,
            tc.tile_pool(name="kt", bufs=4) as ktpool,
            tc.tile_pool(name="e", bufs=1) as epool,
            tc.tile_pool(name="fin", bufs=1) as fpool,
            tc.tile_pool(name="ktp", bufs=4, space="PSUM") as ktppool,
            tc.tile_pool(name="scp", bufs=3, space="PSUM") as scpool,
            tc.tile_pool(name="acc", bufs=1, space="PSUM") as accpool,
        ):
            qt_sb = cpool.tile([D, BH], BF16)
            btt_sb = cpool.tile([GRP, G], I32)
            msk_sb = cpool.tile([GRP, mcols], F8E3)
            zrow = cpool.tile([1, D + 1], F32)
            ident = cpool.tile([128, 128], F8E3)

            nc.sync.dma_start(out=qt_sb[:], in_=qt[:])
            nc.sync.dma_start(out=btt_sb[:], in_=btt[:])
            nc.sync.dma_start(out=msk_sb[:], in_=msk[:])
            nc.vector.memset(zrow[:], 0.0)
            make_identity(nc, ident[:])

            # the gather ring runs PF groups ahead and wraps across reps, so
            # each group's slot (g % NBUF) must be rep-independent: NBUF | G.
            NBUF = max((d for d in range(2, G + 1)
                        if G % d == 0 and d > PF + 1 and d <= 12), default=G)
            xkv_tiles = [xkvpool.tile([GRP, ROW], U8, name=f"xkv{i}",
                                      tag=f"xkv{i}") for i in range(NBUF)]
            e_tiles = [epool.tile([GRP, ECOL], BF16, name=f"e{i}", tag=f"e{i}")
                       for i in range(G)] if do_comp else []
            # No blanket memsets: e/sc tiles are fully rewritten every pass;
            # stale xkv tail rows are finite data from an earlier full group
            # on the same buffer, zeroed by the mask. Only a buffer whose
            # FIRST writer is a partial group needs its tail rows zeroed once
            # (uninitialized SBUF could hold NaN bit patterns).
            first_writer: dict[int, int] = {}
            for g in range(G):
                first_writer.setdefault(g % NBUF, g)
            if do_comp and not do_gather:
                # probe variants without gathers still read xkv tiles
                for t_ in xkv_tiles:
                    nc.vector.memset(t_[:], 0.0)
            if do_gather:
                for ti, g in sorted(first_writer.items()):
                    cnt = key[g][1]
                    if cnt < GRP:
                        # engine APs need a 32-aligned partition base; the
                        # extra rows are rewritten by the first gather anyway
                        p0 = cnt // 32 * 32
                        nc.vector.memset(xkv_tiles[ti][p0:GRP, :], 0.0)

            acc_ps = accpool.tile([BH, D + 1], F32, space="PSUM", tag="acc")

            sink = None
            if not do_comp:
                sink = fpool.tile([GRP, 1], BF16, name="sink", tag="sink")
            kt_stale = None
            if do_comp and not do_trans:
                kt_stale = ktpool.tile([128, 128], F8E3, name="kt_stale", tag="kts")
                nc.vector.memset(kt_stale[:], 0.0)

            # prologue: gathers for the first PF groups (the in-loop gathers
            # run PF groups ahead and wrap to the next rep's start).
            if do_gather:
                for p in range(min(PF, G)):
                    nc.gpsimd.indirect_dma_start(
                        out=xkv_tiles[p % NBUF][:], out_offset=None, in_=kvsh[:],
                        in_offset=bass.IndirectOffsetOnAxis(
                            ap=btt_sb[:, p:p + 1], axis=0),
                        bounds_check=NUM_BLOCKS - 1, oob_is_err=False,
                    )

            import contextlib
            loop_cm = tc.For_i(0, rep, 1) if rep > 1 else contextlib.nullcontext()
            with loop_cm:
                body(nc, tc, key, kvsh, qt_sb, btt_sb, msk_sb, zrow, ident,
                     xkv_tiles, e_tiles, ktpool, ktppool, fpool,
                     scpool, acc_ps, out, sink, kt_stale,
                     do_gather, do_comp, do_trans, use_const)
    nc.compile()
    return nc


def body(nc, tc, key, kvsh, qt_sb, btt_sb, msk_sb, zrow, ident,
         xkv_tiles, e_tiles, ktpool, ktppool, fpool,
         scpool, acc_ps, out, sink, kt_stale, do_gather, do_comp, do_trans,
         use_const=False):
    NBUF = len(xkv_tiles)
    xkv_const = None
    if use_const:
        xkv_const = ktpool.tile([GRP, ROW], U8, name="xkv_const", tag="xkvc")
        nc.vector.memset(xkv_const[:], 0.0)

    def softmax(slab, sc_ps, e, moff_g):
        # e tile is exactly [128, 16*32] — fully rewritten, t-major like msk
        nc.scalar.activation(out=e[:], in_=sc_ps[:],
                             func=mybir.ActivationFunctionType.Exp, scale=SCALE)
        nc.vector.tensor_tensor(out=e[:], in0=e[:],
                                in1=msk_sb[:, moff_g: moff_g + ECOL],
                                op=mybir.AluOpType.mult)

    def pv(slab, e, xkv):
        p0 = slab * SW
        for t in range(BLOCK_SIZE):
            nc.tensor.matmul(
                out=acc_ps[p0:p0 + SW, :],
                lhsT=e[:, t * SW:(t + 1) * SW],
                rhs=xkv[:, VOFF + t * VTOKB:
                        VOFF + t * VTOKB + VTOKB].bitcast(BF16),
                start=False, stop=True, skip_group_check=True,
            )

    moff = 0
    pend = None
    G = len(key)
    if do_comp and G == 1:
        nc.tensor.matmul(out=acc_ps[:], lhsT=zrow[0:1, 0:BH],
                         rhs=zrow[0:1, :], start=True, stop=True,
                         skip_group_check=True)
    for g, (slab, _cnt) in enumerate(key):
        xkv = xkv_tiles[g % NBUF]
        if do_gather:
            gp = (g + PF) % G  # prefetch PF groups ahead (wraps to next rep)
            nc.gpsimd.indirect_dma_start(
                out=xkv_tiles[gp % NBUF][:], out_offset=None, in_=kvsh[:],
                in_offset=bass.IndirectOffsetOnAxis(ap=btt_sb[:, gp:gp + 1], axis=0),
                bounds_check=NUM_BLOCKS - 1, oob_is_err=False,
            )
        if use_const:
            xkv = xkv_const  # decouple compute from the gathers (probe)
        if not do_comp:
            nc.vector.tensor_copy(out=sink[:], in_=xkv[:, 0:1])
            moff += ECOL
            continue

        # PE order per group: 16 transposes (two per PSUM tile, one DVE/ACT
        # copy per pair), then the previous group's 16 PV matmuls (which fill
        # the window while the copies drain), then the 16 scores matmuls --
        # so scores never stall on a copy and the PE stream stays dense. The
        # copies are emitted BEFORE exp/mask(pend) so they lead the ACT/DVE
        # queues (exp must wait for the previous scores anyway).
        sc_ps = scpool.tile([GRP, ECOL], F32, space="PSUM")

        kts = []
        if do_trans:
            for q in range(BLOCK_SIZE // 4):
                # fp8 PE-transpose writes one result byte per 2 PSUM bytes
                # (hw requirement: output element step 2) — use a strided
                # view of a double-width tile, then pack on the copy out.
                ktp = ktppool.tile([128, 1024], F8E3, space="PSUM")
                ktp3 = ktp[:].rearrange("p (c two) -> p c two", two=2)
                for j in range(4):
                    nc.tensor.transpose(
                        out=ktp3[:, j * 128:(j + 1) * 128, 0:1],
                        in_=xkv[:, (4 * q + j) * D:
                                (4 * q + j + 1) * D].bitcast(F8E3),
                        identity=ident[:])
                kt = ktpool.tile([128, 512], F8E3)
                kt3 = kt[:].rearrange("p (c one) -> p c one", one=1)
                if q % 2 == 0:
                    nc.vector.tensor_copy(out=kt3, in_=ktp3[:, :, 0:1])
                else:
                    nc.scalar.copy(out=kt3, in_=ktp3[:, :, 0:1])
                kts.extend(kt[:, j * 128:(j + 1) * 128] for j in range(4))
                if q == 0 and pend is not None:
                    # exp/mask(pend) right after the first copy pair: exp's
                    # input is long ready, so PV(pend) unblocks before the
                    # transposes finish instead of queueing behind copies.
                    softmax(pend[0], pend[1], pend[2], pend[4])
        else:
            kts = [kt_stale[:] for _ in range(BLOCK_SIZE)]

        if pend is not None:
            if not do_trans:
                softmax(pend[0], pend[1], pend[2], pend[4])
            if g == 1 and do_comp:
                # zero the [64, 129] accumulation region here (not at body
                # start) so the first groups' transposes/scores overlap the
                # previous rep's finalization instead of waiting on it.
                nc.tensor.matmul(out=acc_ps[:], lhsT=zrow[0:1, 0:BH],
                                 rhs=zrow[0:1, :], start=True, stop=True,
                                 skip_group_check=True)
            pv(pend[0], pend[2], pend[3])

        for t in range(BLOCK_SIZE):
            nc.tensor.matmul(
                out=sc_ps[:, t * SW:(t + 1) * SW],
                lhsT=kts[t],
                rhs=qt_sb[:, slab * SW: slab * SW + SW],
                start=True, stop=True,
            )

        pend = (slab, sc_ps, e_tiles[g], xkv, moff)
        moff += ECOL

    if do_comp and pend is not None:
        softmax(pend[0], pend[1], pend[2], pend[4])
        pv(pend[0], pend[2], pend[3])

    # ---- finalization ----
    if not do_comp:
        dummy = fpool.tile([BH, D], F32, tag="dummy")
        nc.vector.memset(dummy[:], 0.0)
        nc.sync.dma_start(out=out[:], in_=dummy[:])
        return
    rnorm = fpool.tile([BH, 1], F32, tag="rnorm")
    nc.vector.reciprocal(out=rnorm[:], in_=acc_ps[:, D:D + 1])
    out_sb = fpool.tile([BH, D], F32, tag="out_sb")
    nc.vector.tensor_scalar_mul(out_sb[:], acc_ps[:, 0:D], rnorm[:])
    nc.sync.dma_start(out=out[:], in_=out_sb[:])


def prepare(inputs):
    """Host-side shard prep. Returns (key, per-core in_maps, perm)."""
    q = np.asarray(inputs["q"], dtype=np.float32)
    k = np.asarray(inputs["k"], dtype=np.float32)
    v = np.asarray(inputs["v"], dtype=np.float32)
    k_cache = np.asarray(inputs["k_cache"], dtype=np.float32)
    v_cache = np.asarray(inputs["v_cache"], dtype=np.float32)
    block_tables = np.asarray(inputs["block_tables"], dtype=np.int32)
    context_lens = np.asarray(inputs["context_lens"], dtype=np.int64)
    slot_mapping = np.asarray(inputs["slot_mapping"], dtype=np.int32)

    groups, perm, nb = plan_groups(context_lens)
    key = tuple((slab, cnt) for slab, _, cnt in groups)
    G = len(groups)

    # per-slab concatenated (block, owner-in-slab, position) streams,
    # in permuted batch order
    blocks_sl, owners_sl, pos_sl = [], [], []
    for slab in range(2):
        bs, ow, ps = [], [], []
        for j in range(SLABB):
            b = int(perm[slab * SLABB + j])
            n = int(nb[b])
            bs.append(block_tables[b, :n])
            ow.append(np.full(n, j))
            ps.append(np.arange(n))
        blocks_sl.append(np.concatenate(bs))
        owners_sl.append(np.concatenate(ow))
        pos_sl.append(np.concatenate(ps))

    btt = np.full((GRP, G), OOB, dtype=np.int32)
    msk = np.zeros((GRP, ECOL * G), dtype=ml_dtypes.float8_e3m4)
    cl_perm = context_lens[perm]  # [16] permuted
    for gi, (slab, i0, cnt) in enumerate(groups):
        btt[:cnt, gi] = blocks_sl[slab][i0:i0 + cnt]
        own = owners_sl[slab][i0:i0 + cnt]          # 0..7 within slab
        pos = pos_sl[slab][i0:i0 + cnt]
        moff = ECOL * gi
        for t in range(BLOCK_SIZE):
            tok = pos * BLOCK_SIZE + t
            valid = (tok < cl_perm[slab * SLABB + own]).astype(np.float32)
            for h in range(N_REP):
                cols = moff + t * SW + own * N_REP + h
                msk[np.arange(cnt), cols] = valid
    in_maps = []
    for h in range(KV):
        k_sh = np.ascontiguousarray(k_cache[:, :, h, :]).reshape(-1, D)
        v_sh = np.ascontiguousarray(v_cache[:, :, h, :]).reshape(-1, D)
        k_sh[slot_mapping] = k[:, h, :]
        v_sh[slot_mapping] = v[:, h, :]
        kv = np.zeros((NUM_BLOCKS, ROW), dtype=np.uint8)
        k8 = np.clip(k_sh, -15.0, 15.0).astype(ml_dtypes.float8_e3m4)
        kv[:, :KROW] = k8.view(np.uint8).reshape(NUM_BLOCKS, KROW)
        v_ext = np.zeros((NUM_BLOCKS * BLOCK_SIZE, VTOK), dtype=ml_dtypes.bfloat16)
        v_ext[:, :D] = v_sh
        v_ext[:, D] = 1.0
        kv[:, KROW:] = v_ext.view(np.uint8).reshape(NUM_BLOCKS, VROW)
        # qt columns in permuted batch-major order: col (slabpos)*4 + h
        qp = q.reshape(B, KV, N_REP, D)[perm, h, :, :]   # [16, 4, D] permuted
        qtc = np.ascontiguousarray(qp.transpose(2, 0, 1).reshape(D, BH))
        in_maps.append({
            "kvsh": kv,
            "qt": qtc.astype(ml_dtypes.bfloat16),
            "btt": btt,
            "msk": msk,
        })
    return key, in_maps, perm


def build_program(key: tuple, rep: int = 1):
    return build_program_v(key, rep=rep, variant="full")


def kernel(q, k, v, k_cache, v_cache, block_tables, context_lens, slot_mapping):
    key, in_maps, perm = prepare(dict(
        q=q, k=k, v=v, k_cache=k_cache, v_cache=v_cache,
        block_tables=block_tables, context_lens=context_lens,
        slot_mapping=slot_mapping))

    if key not in _kernel_cache:
        _kernel_cache[key] = build_program(key)
    nc = _kernel_cache[key]

    res = run_bass_kernel_spmd(nc, in_maps, core_ids=list(range(KV)))

    inv = np.argsort(perm)
    full = np.empty((B, NUM_HEADS, D), dtype=np.float32)
    for h in range(KV):
        o = res.results[h]["out"].reshape(B, N_REP, D)[inv]
        full[:, h * N_REP:(h + 1) * N_REP, :] = o
    return full


# revision 18
# speedup vs baseline: 1.3612x; 1.3612x over previous
"""Paged-attention decode kernel for Trainium2, sharded over 8 NeuronCores by KV head.

Problem (hardcoded): B=16, 32 Q heads / 8 KV heads (GQA n_rep=4), D=128,
paged f32 KV cache of 4096 blocks x 16 tokens, 256-entry block tables,
context_lens in [1, 4096), one new (k, v) token per sequence scattered at
slot_mapping before attention.

Strategy per core (1 KV head + its 4 Q heads, all 16 batches):
  host: per-head fused KV shard in a uint8 block-row layout [4096, 6176]:
  row b = [K block b: 16*128 fp8-e3m4 (1B) | V block b: 16*129 bf16 (2B)],
  where each V token's 128 values are followed by a 1.0 (so the PV matmul
  accumulates the softmax denominator free). K in e3m4 (4 mantissa bits,
  |k| <= 5.5 << 15.5 max) costs ~1.5e-2 rel err (< 2e-2 gate) and cuts the
  gathered bytes 25%% -- the kernel is HBM-gather-bound. The one-token
  scatter is applied on host.

  Batches are permuted into two "slabs" of 8 with balanced total block
  counts; each group of <=128 gathered block rows lives entirely in one slab,
  so its 32 query-head columns (8 batches x 4 heads) map to a 32-aligned
  PSUM partition slab (matmul output base must be 0/32/64/96). The host
  unpermutes batches when reassembling the output.

  device, per group g (fp8/bf16 on PE => 1 cycle/row + fast weight load):
    ONE indirect DMA gathers the group's fused KV block rows (8.2KB per
    partition; padded slots bounds-skipped).
    PE emits all 16 transposes xkv[:, t*128:(t+1)*128] -> ktp [d, blk]
    (PSUM bf16) first; DVE/ACT alternate copying them to SBUF while PE
    streams the remaining transposes, so the 16 scores matmuls (lhsT=kt,
    rhs=qt[d, 32 slab cols] -> [blk, 32] PSUM) never stall. One ACT exp
    (scale folded) writes the group's e tile [blk, (t=16, 32)] IN FULL; one
    DVE mask-mult zeroes invalid tokens / padded blocks / non-owning
    batches. For t: matmul(out=acc[slab*32:+32, 0:129], lhsT=e[:, t*32:
    (t+1)*32], rhs=xkv[:, VOFF + t*129 :+129]) accumulates out + den.
    No SBUF memsets in the prologue: e/sc tiles are fully rewritten every
    pass, and stale xkv tail rows are finite data zeroed by the mask (only
    a buffer whose FIRST writer is a partial group gets a one-time memset
    of its tail rows).
  final: out = acc[:, :128] * reciprocal(acc[:, 128]); DMA out (rows in
  permuted batch-major order; host unpermutes).

The Bass program is JIT-specialized on the group structure (a small tuple
derived from context_lens); compiled programs are cached per structure.
"""

import numpy as np
import ml_dtypes

import concourse.bass as bass
import concourse.bacc as bacc
import concourse.mybir as mybir
import concourse.tile as tile
from concourse.bass_utils import run_bass_kernel_spmd
from concourse.masks import make_identity

F32 = mybir.dt.float32
BF16 = mybir.dt.bfloat16
F8E3 = mybir.dt.float8e3
U8 = mybir.dt.uint8
I32 = mybir.dt.int32

B = 16
NUM_HEADS = 32
KV = 8
D = 128
SCALE = 0.08838834764831845
NUM_BLOCKS = 4096
BLOCK_SIZE = 16
BLOCKS_PER_SEQ = 256
N_REP = NUM_HEADS // KV   # 4
BH = B * N_REP            # 64
GRP = 128                 # blocks per group (one per partition)
SLABB = 8                 # batches per slab (8*4 heads = 32 partitions)
SW = SLABB * N_REP        # 32: query columns per slab
KROW = BLOCK_SIZE * D           # 2048 B: K part of a row (fp8 e3m4)
VTOK = D + 1                    # 129: 128 v + 1.0 (bf16)
VTOKB = VTOK * 2                # 258 B per V token
VROW = BLOCK_SIZE * VTOKB       # 4128 B: V part of a row
ROW = KROW + VROW               # 6176 B fused row (uint8-typed)
VOFF = KROW                     # V byte offset within a row
ECOL = BLOCK_SIZE * SW          # 512 cols per e tile
OOB = 8192                      # block-table marker for padded slots
PF = 7                          # gather prefetch distance (groups)

_kernel_cache: dict = {}


def plan_batches(context_lens):
    """Permute batches into two 8-batch slabs with balanced block counts."""
    nb = np.maximum(1, -(-np.asarray(context_lens).astype(np.int64) // BLOCK_SIZE))
    nb = np.minimum(nb, BLOCKS_PER_SEQ)
    order = np.argsort(-nb, kind="stable")
    sets = ([], [])
    sums = [0, 0]
    for b in order:
        i = 0 if (sums[0] <= sums[1] and len(sets[0]) < SLABB) or \
                 len(sets[1]) >= SLABB else 1
        sets[i].append(int(b))
        sums[i] += int(nb[b])
    perm = np.array(sorted(sets[0]) + sorted(sets[1]), dtype=np.int64)
    return perm, nb


def plan_groups(context_lens):
    perm, nb = plan_batches(context_lens)
    groups = []       # (slab, start_in_slab, cnt)
    for slab in range(2):
        tot = int(sum(nb[perm[slab * SLABB + j]] for j in range(SLABB)))
        i = 0
        while i < tot:
            c = min(GRP, tot - i)
            groups.append((slab, i, c))
            i += c
    return groups, perm, nb


def build_program_v(key: tuple, rep: int = 1, variant: str = "full"):
    """key = tuple of (slab, cnt) per group."""
    G = len(key)
    mcols = ECOL * G

    nc = bacc.Bacc("TRN2", target_bir_lowering=False, debug=False)
    kvsh = nc.dram_tensor("kvsh", [NUM_BLOCKS, ROW], U8, kind="ExternalInput")
    qt = nc.dram_tensor("qt", [D, BH], BF16, kind="ExternalInput")
    btt = nc.dram_tensor("btt", [GRP, G], I32, kind="ExternalInput")
    msk = nc.dram_tensor("msk", [GRP, mcols], F8E3, kind="ExternalInput")
    out = nc.dram_tensor("out", [BH, D], F32, kind="ExternalOutput")

    do_gather = variant in ("full", "dma", "overlap")
    do_comp = variant in ("full", "compute", "notrans", "overlap")
    do_trans = variant in ("full", "compute", "overlap")
    use_const = variant == "overlap"

    with tile.TileContext(nc) as tc:
        with (
            tc.tile_pool(name="const", bufs=1) as cpool,
            tc.tile_pool(name="xkv", bufs=1) as xkvpool,
            tc.tile_pool(name="kt", bufs=4) as ktpool,
            tc.tile_pool(name="e", bufs=1) as epool,
            tc.tile_pool(name="fin", bufs=1) as fpool,
            tc.tile_pool(name="ktp", bufs=4, space="PSUM") as ktppool,
            tc.tile_pool(name="scp", bufs=3, space="PSUM") as scpool,
            tc.tile_pool(name="acc", bufs=1, space="PSUM") as accpool,
        ):
            qt_sb = cpool.tile([D, BH], BF16)
            btt_sb = cpool.tile([GRP, G], I32)
            msk_sb = cpool.tile([GRP, mcols], F8E3)
            zrow = cpool.tile([1, D + 1], F32)
            ident = cpool.tile([128, 128], F8E3)

            nc.sync.dma_start(out=qt_sb[:], in_=qt[:])
            nc.sync.dma_start(out=btt_sb[:], in_=btt[:])
            nc.sync.dma_start(out=msk_sb[:], in_=msk[:])
            nc.vector.memset(zrow[:], 0.0)
            make_identity(nc, ident[:])

            # the gather ring runs PF groups ahead and wraps across reps, so
            # each group's slot (g % NBUF) must be rep-independent: NBUF | G.
            NBUF = max((d for d in range(2, G + 1)
                        if G % d == 0 and d > PF + 1 and d <= 12), default=G)
            xkv_tiles = [xkvpool.tile([GRP, ROW], U8, name=f"xkv{i}",
                                      tag=f"xkv{i}") for i in range(NBUF)]
            e_tiles = [epool.tile([GRP, ECOL], BF16, name=f"e{i}", tag=f"e{i}")
                       for i in range(G)] if do_comp else []
            # No blanket memsets: e/sc tiles are fully rewritten every pass;
            # stale xkv tail rows are finite data from an earlier full group
            # on the same buffer, zeroed by the mask. Only a buffer whose
            # FIRST writer is a partial group needs its tail rows zeroed once
            # (uninitialized SBUF could hold NaN bit patterns).
            first_writer: dict[int, int] = {}
            for g in range(G):
                first_writer.setdefault(g % NBUF, g)
            if do_comp and not do_gather:
                # probe variants without gathers still read xkv tiles
                for t_ in xkv_tiles:
                    nc.vector.memset(t_[:], 0.0)
            if do_gather:
                for ti, g in sorted(first_writer.items()):
                    cnt = key[g][1]
                    if cnt < GRP:
                        # engine APs need a 32-aligned partition base; the
                        # extra rows are rewritten by the first gather anyway
                        p0 = cnt // 32 * 32
                        nc.vector.memset(xkv_tiles[ti][p0:GRP, :], 0.0)

            acc_ps = accpool.tile([BH, D + 1], F32, space="PSUM", tag="acc")

            sink = None
            if not do_comp:
                sink = fpool.tile([GRP, 1], BF16, name="sink", tag="sink")
            kt_stale = None
            if do_comp and not do_trans:
                kt_stale = ktpool.tile([128, 128], F8E3, name="kt_stale", tag="kts")
                nc.vector.memset(kt_stale[:], 0.0)

            # prologue: gathers for the first PF groups (the in-loop gathers
            # run PF groups ahead and wrap to the next rep's start).
            if do_gather:
                for p in range(min(PF, G)):
                    nc.gpsimd.indirect_dma_start(
                        out=xkv_tiles[p % NBUF][:], out_offset=None, in_=kvsh[:],
                        in_offset=bass.IndirectOffsetOnAxis(
                            ap=btt_sb[:, p:p + 1], axis=0),
                        bounds_check=NUM_BLOCKS - 1, oob_is_err=False,
                    )

            import contextlib
            loop_cm = tc.For_i(0, rep, 1) if rep > 1 else contextlib.nullcontext()
            with loop_cm:
                body(nc, tc, key, kvsh, qt_sb, btt_sb, msk_sb, zrow, ident,
                     xkv_tiles, e_tiles, ktpool, ktppool, fpool,
                     scpool, acc_ps, out, sink, kt_stale,
                     do_gather, do_comp, do_trans, use_const)
    nc.compile()
    return nc


def body(nc, tc, key, kvsh, qt_sb, btt_sb, msk_sb, zrow, ident,
         xkv_tiles, e_tiles, ktpool, ktppool, fpool,
         scpool, acc_ps, out, sink, kt_stale, do_gather, do_comp, do_trans,
         use_const=False):
    NBUF = len(xkv_tiles)
    xkv_const = None
    if use_const:
        xkv_const = ktpool.tile([GRP, ROW], U8, name="xkv_const", tag="xkvc")
        nc.vector.memset(xkv_const[:], 0.0)

    def softmax(slab, sc_ps, e, moff_g):
        # e tile is exactly [128, 16*32] — fully rewritten, t-major like msk
        nc.scalar.activation(out=e[:], in_=sc_ps[:],
                             func=mybir.ActivationFunctionType.Exp, scale=SCALE)
        nc.vector.tensor_tensor(out=e[:], in0=e[:],
                                in1=msk_sb[:, moff_g: moff_g + ECOL],
                                op=mybir.AluOpType.mult)

    def pv(slab, e, xkv):
        p0 = slab * SW
        for t in range(BLOCK_SIZE):
            nc.tensor.matmul(
                out=acc_ps[p0:p0 + SW, :],
                lhsT=e[:, t * SW:(t + 1) * SW],
                rhs=xkv[:, VOFF + t * VTOKB:
                        VOFF + t * VTOKB + VTOKB].bitcast(BF16),
                start=False, stop=True, skip_group_check=True,
            )

    moff = 0
    pend = None
    G = len(key)
    if do_comp and G == 1:
        nc.tensor.matmul(out=acc_ps[:], lhsT=zrow[0:1, 0:BH],
                         rhs=zrow[0:1, :], start=True, stop=True,
                         skip_group_check=True)
    for g, (slab, _cnt) in enumerate(key):
        xkv = xkv_tiles[g % NBUF]
        if do_gather:
            gp = (g + PF) % G  # prefetch PF groups ahead (wraps to next rep)
            nc.gpsimd.indirect_dma_start(
                out=xkv_tiles[gp % NBUF][:], out_offset=None, in_=kvsh[:],
                in_offset=bass.IndirectOffsetOnAxis(ap=btt_sb[:, gp:gp + 1], axis=0),
                bounds_check=NUM_BLOCKS - 1, oob_is_err=False,
            )
        if use_const:
            xkv = xkv_const  # decouple compute from the gathers (probe)
        if not do_comp:
            nc.vector.tensor_copy(out=sink[:], in_=xkv[:, 0:1])
            moff += ECOL
            continue

        # PE order per group: 16 transposes (two per PSUM tile, one DVE/ACT
        # copy per pair), then the previous group's 16 PV matmuls (which fill
        # the window while the copies drain), then the 16 scores matmuls --
        # so scores never stall on a copy and the PE stream stays dense. The
        # copies are emitted BEFORE exp/mask(pend) so they lead the ACT/DVE
        # queues (exp must wait for the previous scores anyway).
        sc_ps = scpool.tile([GRP, ECOL], F32, space="PSUM")

        kts = []
        if do_trans:
            for q in range(BLOCK_SIZE // 4):
                # fp8 PE-transpose writes one result byte per 2 PSUM bytes
                # (hw requirement: output element step 2) — use a strided
                # view of a double-width tile, then pack on the copy out.
                ktp = ktppool.tile([128, 1024], F8E3, space="PSUM")
                ktp3 = ktp[:].rearrange("p (c two) -> p c two", two=2)
                for j in range(4):
                    nc.tensor.transpose(
                        out=ktp3[:, j * 128:(j + 1) * 128, 0:1],
                        in_=xkv[:, (4 * q + j) * D:
                                (4 * q + j + 1) * D].bitcast(F8E3),
                        identity=ident[:])
                kt = ktpool.tile([128, 512], F8E3)
                kt3 = kt[:].rearrange("p (c one) -> p c one", one=1)
                if q % 2 == 0:
                    nc.vector.tensor_copy(out=kt3, in_=ktp3[:, :, 0:1])
                else:
                    nc.scalar.copy(out=kt3, in_=ktp3[:, :, 0:1])
                kts.extend(kt[:, j * 128:(j + 1) * 128] for j in range(4))
                if q == 0 and pend is not None:
                    # exp/mask(pend) right after the first copy pair: exp's
                    # input is long ready, so PV(pend) unblocks before the
                    # transposes finish instead of queueing behind copies.
                    softmax(pend[0], pend[1], pend[2], pend[4])
        else:
            kts = [kt_stale[:] for _ in range(BLOCK_SIZE)]

        if pend is not None:
            if not do_trans:
                softmax(pend[0], pend[1], pend[2], pend[4])
            if g == 1 and do_comp:
                # zero the [64, 129] accumulation region here (not at body
                # start) so the first groups' transposes/scores overlap the
                # previous rep's finalization instead of waiting on it.
                nc.tensor.matmul(out=acc_ps[:], lhsT=zrow[0:1, 0:BH],
                                 rhs=zrow[0:1, :], start=True, stop=True,
                                 skip_group_check=True)
            pv(pend[0], pend[2], pend[3])

        for t in range(BLOCK_SIZE):
            nc.tensor.matmul(
                out=sc_ps[:, t * SW:(t + 1) * SW],
                lhsT=kts[t],
                rhs=qt_sb[:, slab * SW: slab * SW + SW],
                start=True, stop=True,
            )

        pend = (slab, sc_ps, e_tiles[g], xkv, moff)
        moff += ECOL

    if do_comp and pend is not None:
        softmax(pend[0], pend[1], pend[2], pend[4])
        pv(pend[0], pend[2], pend[3])

    # ---- finalization ----
    if not do_comp:
        dummy = fpool.tile([BH, D], F32, tag="dummy")
        nc.vector.memset(dummy[:], 0.0)
        nc.sync.dma_start(out=out[:], in_=dummy[:])
        return
    rnorm = fpool.tile([BH, 1], F32, tag="rnorm")
    nc.vector.reciprocal(out=rnorm[:], in_=acc_ps[:, D:D + 1])
    out_sb = fpool.tile([BH, D], F32, tag="out_sb")
    nc.vector.tensor_scalar_mul(out_sb[:], acc_ps[:, 0:D], rnorm[:])
    nc.sync.dma_start(out=out[:], in_=out_sb[:])


def prepare(inputs):
    """Host-side shard prep. Returns (key, per-core in_maps, perm)."""
    q = np.asarray(inputs["q"], dtype=np.float32)
    k = np.asarray(inputs["k"], dtype=np.float32)
    v = np.asarray(inputs["v"], dtype=np.float32)
    k_cache = np.asarray(inputs["k_cache"], dtype=np.float32)
    v_cache = np.asarray(inputs["v_cache"], dtype=np.float32)
    block_tables = np.asarray(inputs["block_tables"], dtype=np.int32)
    context_lens = np.asarray(inputs["context_lens"], dtype=np.int64)
    slot_mapping = np.asarray(inputs["slot_mapping"], dtype=np.int32)

    perm, nb = plan_batches(context_lens)
    cl_perm = context_lens[perm]  # [16] permuted

    # Dedup within each slab: a block row gathered once can serve several
    # (batch, position) occurrences — scores are computed for all 32 slab
    # columns anyway, so multiplicity/ownership lives entirely in the mask
    # (weight = occurrence count; duplicate blocks in the reference sum
    # twice, so a merged row with weight 2 is numerically identical).
    # Only fully-valid occurrences merge; tail blocks (partial validity)
    # stay as individual rows with per-token masks.
    rows_sl = []
    for slab in range(2):
        merged: dict = {}
        order = []
        tails = []
        for j in range(SLABB):
            b = int(perm[slab * SLABB + j])
            n = int(nb[b])
            ctx = int(context_lens[b])
            for p in range(n):
                blk = int(block_tables[b, p])
                if (p + 1) * BLOCK_SIZE > ctx:   # tail block: partial tokens
                    tails.append((blk, j, p))
                else:
                    if blk not in merged:
                        merged[blk] = {}
                        order.append(blk)
                    merged[blk][j] = merged[blk].get(j, 0) + 1
        rows = [(blk, merged[blk], None) for blk in order]
        rows += [(blk, {j: 1}, (j, p)) for blk, j, p in tails]
        rows_sl.append(rows)

    groups = []
    for slab in range(2):
        tot = len(rows_sl[slab])
        i = 0
        while i < tot:
            c = min(GRP, tot - i)
            groups.append((slab, i, c))
            i += c
    key = tuple((slab, cnt) for slab, _, cnt in groups)
    G = len(groups)

    btt = np.full((GRP, G), OOB, dtype=np.int32)
    msk = np.zeros((GRP, ECOL * G), dtype=np.float32)
    for gi, (slab, i0, cnt) in enumerate(groups):
        moff = ECOL * gi
        for r in range(cnt):
            blk, owners, tail = rows_sl[slab][i0 + r]
            btt[r, gi] = blk
            if tail is None:
                for j, c in owners.items():
                    base = moff + j * N_REP
                    for t in range(BLOCK_SIZE):
                        msk[r, base + t * SW: base + t * SW + N_REP] = c
            else:
                (j, p) = tail
                ctx = int(cl_perm[slab * SLABB + j])
                base = moff + j * N_REP
                for t in range(BLOCK_SIZE):
                    if p * BLOCK_SIZE + t < ctx:
                        msk[r, base + t * SW: base + t * SW + N_REP] = 1.0
    msk = msk.astype(ml_dtypes.float8_e3m4)
    in_maps = []
    for h in range(KV):
        k_sh = np.ascontiguousarray(k_cache[:, :, h, :]).reshape(-1, D)
        v_sh = np.ascontiguousarray(v_cache[:, :, h, :]).reshape(-1, D)
        k_sh[slot_mapping] = k[:, h, :]
        v_sh[slot_mapping] = v[:, h, :]
        kv = np.zeros((NUM_BLOCKS, ROW), dtype=np.uint8)
        k8 = np.clip(k_sh, -15.0, 15.0).astype(ml_dtypes.float8_e3m4)
        kv[:, :KROW] = k8.view(np.uint8).reshape(NUM_BLOCKS, KROW)
        v_ext = np.zeros((NUM_BLOCKS * BLOCK_SIZE, VTOK), dtype=ml_dtypes.bfloat16)
        v_ext[:, :D] = v_sh
        v_ext[:, D] = 1.0
        kv[:, KROW:] = v_ext.view(np.uint8).reshape(NUM_BLOCKS, VROW)
        # qt columns in permuted batch-major order: col (slabpos)*4 + h
        qp = q.reshape(B, KV, N_REP, D)[perm, h, :, :]   # [16, 4, D] permuted
        qtc = np.ascontiguousarray(qp.transpose(2, 0, 1).reshape(D, BH))
        in_maps.append({
            "kvsh": kv,
            "qt": qtc.astype(ml_dtypes.bfloat16),
            "btt": btt,
            "msk": msk,
        })
    return key, in_maps, perm


def build_program(key: tuple, rep: int = 1):
    return build_program_v(key, rep=rep, variant="full")


def kernel(q, k, v, k_cache, v_cache, block_tables, context_lens, slot_mapping):
    key, in_maps, perm = prepare(dict(
        q=q, k=k, v=v, k_cache=k_cache, v_cache=v_cache,
        block_tables=block_tables, context_lens=context_lens,
        slot_mapping=slot_mapping))

    if key not in _kernel_cache:
        _kernel_cache[key] = build_program(key)
    nc = _kernel_cache[key]

    res = run_bass_kernel_spmd(nc, in_maps, core_ids=list(range(KV)))

    inv = np.argsort(perm)
    full = np.empty((B, NUM_HEADS, D), dtype=np.float32)
    for h in range(KV):
        o = res.results[h]["out"].reshape(B, N_REP, D)[inv]
        full[:, h * N_REP:(h + 1) * N_REP, :] = o
    return full


# revision 20
# speedup vs baseline: 1.8168x; 1.3347x over previous
"""Paged-attention decode kernel for Trainium2, sharded over 8 NeuronCores by KV head.

Problem (hardcoded): B=16, 32 Q heads / 8 KV heads (GQA n_rep=4), D=128,
paged f32 KV cache of 4096 blocks x 16 tokens, 256-entry block tables,
context_lens in [1, 4096), one new (k, v) token per sequence scattered at
slot_mapping before attention.

Strategy per core (1 KV head + its 4 Q heads, all 16 batches):
  host: per-head fused KV shard in a uint8 block-row layout [4096, 6176]:
  row b = [K block b: 16*128 fp8-e3m4 (1B) | V block b: 16*129 bf16 (2B)],
  where each V token's 128 values are followed by a 1.0 (so the PV matmul
  accumulates the softmax denominator free). K in e3m4 (4 mantissa bits,
  |k| <= 5.5 << 15.5 max) costs ~1.5e-2 rel err (< 2e-2 gate) and cuts the
  gathered bytes 25%% -- the kernel is HBM-gather-bound. The one-token
  scatter is applied on host.

  Batches are permuted into two "slabs" of 8 with balanced total block
  counts; each group of <=128 gathered block rows lives entirely in one slab,
  so its 32 query-head columns (8 batches x 4 heads) map to a 32-aligned
  PSUM partition slab (matmul output base must be 0/32/64/96). The host
  unpermutes batches when reassembling the output.

  device, per group g (fp8/bf16 on PE => 1 cycle/row + fast weight load):
    ONE indirect DMA gathers the group's fused KV block rows (8.2KB per
    partition; padded slots bounds-skipped).
    PE emits all 16 transposes xkv[:, t*128:(t+1)*128] -> ktp [d, blk]
    (PSUM bf16) first; DVE/ACT alternate copying them to SBUF while PE
    streams the remaining transposes, so the 16 scores matmuls (lhsT=kt,
    rhs=qt[d, 32 slab cols] -> [blk, 32] PSUM) never stall. One ACT exp
    (scale folded) writes the group's e tile [blk, (t=16, 32)] IN FULL; one
    DVE mask-mult zeroes invalid tokens / padded blocks / non-owning
    batches. For t: matmul(out=acc[slab*32:+32, 0:129], lhsT=e[:, t*32:
    (t+1)*32], rhs=xkv[:, VOFF + t*129 :+129]) accumulates out + den.
    No SBUF memsets in the prologue: e/sc tiles are fully rewritten every
    pass, and stale xkv tail rows are finite data zeroed by the mask (only
    a buffer whose FIRST writer is a partial group gets a one-time memset
    of its tail rows).
  final: out = acc[:, :128] * reciprocal(acc[:, 128]); DMA out (rows in
  permuted batch-major order; host unpermutes).

The Bass program is JIT-specialized on the group structure (a small tuple
derived from context_lens); compiled programs are cached per structure.
"""

import numpy as np
import ml_dtypes

import concourse.bass as bass
import concourse.bacc as bacc
import concourse.mybir as mybir
import concourse.tile as tile
from concourse.bass_utils import run_bass_kernel_spmd
from concourse.masks import make_identity

F32 = mybir.dt.float32
BF16 = mybir.dt.bfloat16
F8E3 = mybir.dt.float8e3
U8 = mybir.dt.uint8
I32 = mybir.dt.int32

B = 16
NUM_HEADS = 32
KV = 8
D = 128
SCALE = 0.08838834764831845
NUM_BLOCKS = 4096
BLOCK_SIZE = 16
BLOCKS_PER_SEQ = 256
N_REP = NUM_HEADS // KV   # 4
BH = B * N_REP            # 64
GRP = 128                 # blocks per group (one per partition)
SLABB = 8                 # batches per slab (8*4 heads = 32 partitions)
SW = SLABB * N_REP        # 32: query columns per slab
KROW = BLOCK_SIZE * D           # 2048 B: K part of a row (fp8 e3m4)
VTOK = D + 1                    # 129: 128 v + 1.0 (bf16)
VTOKB = VTOK * 2                # 258 B per V token
VROW = BLOCK_SIZE * VTOKB       # 4128 B: V part of a row
ROW = KROW + VROW               # 6176 B fused row (uint8-typed)
VOFF = KROW                     # V byte offset within a row
ECOL = BLOCK_SIZE * SW          # 512 cols per e tile
OOB = 8192                      # block-table marker for padded slots
PF = 7                          # gather prefetch distance (groups)

_kernel_cache: dict = {}


def plan_batches(context_lens):
    """Permute batches into two 8-batch slabs with balanced block counts."""
    nb = np.maximum(1, -(-np.asarray(context_lens).astype(np.int64) // BLOCK_SIZE))
    nb = np.minimum(nb, BLOCKS_PER_SEQ)
    order = np.argsort(-nb, kind="stable")
    sets = ([], [])
    sums = [0, 0]
    for b in order:
        i = 0 if (sums[0] <= sums[1] and len(sets[0]) < SLABB) or \
                 len(sets[1]) >= SLABB else 1
        sets[i].append(int(b))
        sums[i] += int(nb[b])
    perm = np.array(sorted(sets[0]) + sorted(sets[1]), dtype=np.int64)
    return perm, nb


def plan_groups(context_lens):
    perm, nb = plan_batches(context_lens)
    groups = []       # (slab, start_in_slab, cnt)
    for slab in range(2):
        tot = int(sum(nb[perm[slab * SLABB + j]] for j in range(SLABB)))
        i = 0
        while i < tot:
            c = min(GRP, tot - i)
            groups.append((slab, i, c))
            i += c
    return groups, perm, nb


def build_program_v(key: tuple, rep: int = 1, variant: str = "full"):
    """key = tuple of (slab, cnt) per group."""
    G = len(key)
    mcols = ECOL * G

    nc = bacc.Bacc("TRN2", target_bir_lowering=False, debug=False)
    kvsh = nc.dram_tensor("kvsh", [NUM_BLOCKS, ROW], U8, kind="ExternalInput")
    qt = nc.dram_tensor("qt", [D, BH], BF16, kind="ExternalInput")
    btt = nc.dram_tensor("btt", [GRP, G], I32, kind="ExternalInput")
    msk = nc.dram_tensor("msk", [GRP, mcols], F8E3, kind="ExternalInput")
    out = nc.dram_tensor("out", [BH, D], F32, kind="ExternalOutput")

    do_gather = variant in ("full", "dma", "overlap")
    do_comp = variant in ("full", "compute", "notrans", "overlap")
    do_trans = variant in ("full", "compute", "overlap")
    use_const = variant == "overlap"

    with tile.TileContext(nc) as tc:
        with (
            tc.tile_pool(name="const", bufs=1) as cpool,
            tc.tile_pool(name="xkv", bufs=1) as xkvpool,
            tc.tile_pool(name="kt", bufs=4) as ktpool,
            tc.tile_pool(name="e", bufs=1) as epool,
            tc.tile_pool(name="fin", bufs=1) as fpool,
            tc.tile_pool(name="ktp", bufs=4, space="PSUM") as ktppool,
            tc.tile_pool(name="scp", bufs=3, space="PSUM") as scpool,
            tc.tile_pool(name="acc", bufs=1, space="PSUM") as accpool,
        ):
            qt_sb = cpool.tile([D, BH], BF16)
            btt_sb = cpool.tile([GRP, G], I32)
            msk_sb = cpool.tile([GRP, mcols], F8E3)
            zrow = cpool.tile([1, D + 1], F32)
            ident = cpool.tile([128, 128], F8E3)

            nc.sync.dma_start(out=qt_sb[:], in_=qt[:])
            nc.sync.dma_start(out=btt_sb[:], in_=btt[:])
            nc.sync.dma_start(out=msk_sb[:], in_=msk[:])
            nc.vector.memset(zrow[:], 0.0)
            make_identity(nc, ident[:])

            # the gather ring runs PF groups ahead and wraps across reps, so
            # each group's slot (g % NBUF) must be rep-independent: NBUF | G.
            NBUF = max((d for d in range(2, G + 1)
                        if G % d == 0 and d > PF + 1 and d <= 12), default=G)
            xkv_tiles = [xkvpool.tile([GRP, ROW], U8, name=f"xkv{i}",
                                      tag=f"xkv{i}") for i in range(NBUF)]
            e_tiles = [epool.tile([GRP, ECOL], BF16, name=f"e{i}", tag=f"e{i}")
                       for i in range(G)] if do_comp else []
            # No blanket memsets: e/sc tiles are fully rewritten every pass;
            # stale xkv tail rows are finite data from an earlier full group
            # on the same buffer, zeroed by the mask. Only a buffer whose
            # FIRST writer is a partial group needs its tail rows zeroed once
            # (uninitialized SBUF could hold NaN bit patterns).
            first_writer: dict[int, int] = {}
            for g in range(G):
                first_writer.setdefault(g % NBUF, g)
            if do_comp and not do_gather:
                # probe variants without gathers still read xkv tiles
                for t_ in xkv_tiles:
                    nc.vector.memset(t_[:], 0.0)
            if do_gather:
                for ti, g in sorted(first_writer.items()):
                    cnt = key[g][1]
                    if cnt < GRP:
                        # engine APs need a 32-aligned partition base; the
                        # extra rows are rewritten by the first gather anyway
                        p0 = cnt // 32 * 32
                        nc.vector.memset(xkv_tiles[ti][p0:GRP, :], 0.0)

            acc_ps = accpool.tile([BH, D + 1], F32, space="PSUM", tag="acc")

            sink = None
            if not do_comp:
                sink = fpool.tile([GRP, 1], BF16, name="sink", tag="sink")
            kt_stale = None
            if do_comp and not do_trans:
                kt_stale = ktpool.tile([128, 128], F8E3, name="kt_stale", tag="kts")
                nc.vector.memset(kt_stale[:], 0.0)

            # prologue: gathers for the first PF groups (the in-loop gathers
            # run PF groups ahead and wrap to the next rep's start).
            if do_gather:
                for p in range(min(PF, G)):
                    nc.gpsimd.indirect_dma_start(
                        out=xkv_tiles[p % NBUF][:], out_offset=None, in_=kvsh[:],
                        in_offset=bass.IndirectOffsetOnAxis(
                            ap=btt_sb[:, p:p + 1], axis=0),
                        bounds_check=NUM_BLOCKS - 1, oob_is_err=False,
                    )

            import contextlib
            loop_cm = tc.For_i(0, rep, 1) if rep > 1 else contextlib.nullcontext()
            with loop_cm:
                body(nc, tc, key, kvsh, qt_sb, btt_sb, msk_sb, zrow, ident,
                     xkv_tiles, e_tiles, ktpool, ktppool, fpool,
                     scpool, acc_ps, out, sink, kt_stale,
                     do_gather, do_comp, do_trans, use_const)
    nc.compile()
    return nc


def body(nc, tc, key, kvsh, qt_sb, btt_sb, msk_sb, zrow, ident,
         xkv_tiles, e_tiles, ktpool, ktppool, fpool,
         scpool, acc_ps, out, sink, kt_stale, do_gather, do_comp, do_trans,
         use_const=False):
    NBUF = len(xkv_tiles)
    xkv_const = None
    if use_const:
        xkv_const = ktpool.tile([GRP, ROW], U8, name="xkv_const", tag="xkvc")
        nc.vector.memset(xkv_const[:], 0.0)

    def softmax(slab, sc_ps, e, moff_g):
        # e tile is exactly [128, 16*32] — fully rewritten, t-major like msk
        nc.scalar.activation(out=e[:], in_=sc_ps[:],
                             func=mybir.ActivationFunctionType.Exp, scale=SCALE)
        nc.vector.tensor_tensor(out=e[:], in0=e[:],
                                in1=msk_sb[:, moff_g: moff_g + ECOL],
                                op=mybir.AluOpType.mult)

    def pv(slab, e, xkv):
        p0 = slab * SW
        for t in range(BLOCK_SIZE):
            nc.tensor.matmul(
                out=acc_ps[p0:p0 + SW, :],
                lhsT=e[:, t * SW:(t + 1) * SW],
                rhs=xkv[:, VOFF + t * VTOKB:
                        VOFF + t * VTOKB + VTOKB].bitcast(BF16),
                start=False, stop=True, skip_group_check=True,
            )

    moff = 0
    pend = None
    G = len(key)
    if do_comp and G == 1:
        nc.tensor.matmul(out=acc_ps[:], lhsT=zrow[0:1, 0:BH],
                         rhs=zrow[0:1, :], start=True, stop=True,
                         skip_group_check=True)
    for g, (slab, _cnt) in enumerate(key):
        xkv = xkv_tiles[g % NBUF]
        if do_gather:
            gp = (g + PF) % G  # prefetch PF groups ahead (wraps to next rep)
            nc.gpsimd.indirect_dma_start(
                out=xkv_tiles[gp % NBUF][:], out_offset=None, in_=kvsh[:],
                in_offset=bass.IndirectOffsetOnAxis(ap=btt_sb[:, gp:gp + 1], axis=0),
                bounds_check=NUM_BLOCKS - 1, oob_is_err=False,
            )
        if use_const:
            xkv = xkv_const  # decouple compute from the gathers (probe)
        if not do_comp:
            nc.vector.tensor_copy(out=sink[:], in_=xkv[:, 0:1])
            moff += ECOL
            continue

        # PE order per group: 16 transposes (two per PSUM tile, one DVE/ACT
        # copy per pair), then the previous group's 16 PV matmuls (which fill
        # the window while the copies drain), then the 16 scores matmuls --
        # so scores never stall on a copy and the PE stream stays dense. The
        # copies are emitted BEFORE exp/mask(pend) so they lead the ACT/DVE
        # queues (exp must wait for the previous scores anyway).
        sc_ps = scpool.tile([GRP, ECOL], F32, space="PSUM")

        kts = []
        if do_trans:
            for q in range(BLOCK_SIZE // 4):
                # fp8 PE-transpose writes one result byte per 2 PSUM bytes
                # (hw requirement: output element step 2) — use a strided
                # view of a double-width tile, then pack on the copy out.
                ktp = ktppool.tile([128, 1024], F8E3, space="PSUM")
                ktp3 = ktp[:].rearrange("p (c two) -> p c two", two=2)
                for j in range(4):
                    nc.tensor.transpose(
                        out=ktp3[:, j * 128:(j + 1) * 128, 0:1],
                        in_=xkv[:, (4 * q + j) * D:
                                (4 * q + j + 1) * D].bitcast(F8E3),
                        identity=ident[:])
                kt = ktpool.tile([128, 512], F8E3)
                kt3 = kt[:].rearrange("p (c one) -> p c one", one=1)
                if q % 2 == 0:
                    nc.vector.tensor_copy(out=kt3, in_=ktp3[:, :, 0:1])
                else:
                    nc.scalar.copy(out=kt3, in_=ktp3[:, :, 0:1])
                kts.extend(kt[:, j * 128:(j + 1) * 128] for j in range(4))
                if q == 0 and pend is not None:
                    # exp/mask(pend) right after the first copy pair: exp's
                    # input is long ready, so PV(pend) unblocks before the
                    # transposes finish instead of queueing behind copies.
                    softmax(pend[0], pend[1], pend[2], pend[4])
        else:
            kts = [kt_stale[:] for _ in range(BLOCK_SIZE)]

        if pend is not None:
            if not do_trans:
                softmax(pend[0], pend[1], pend[2], pend[4])
            if g == 1 and do_comp:
                # zero the [64, 129] accumulation region here (not at body
                # start) so the first groups' transposes/scores overlap the
                # previous rep's finalization instead of waiting on it.
                nc.tensor.matmul(out=acc_ps[:], lhsT=zrow[0:1, 0:BH],
                                 rhs=zrow[0:1, :], start=True, stop=True,
                                 skip_group_check=True)
            pv(pend[0], pend[2], pend[3])

        for t in range(BLOCK_SIZE):
            nc.tensor.matmul(
                out=sc_ps[:, t * SW:(t + 1) * SW],
                lhsT=kts[t],
                rhs=qt_sb[:, slab * SW: slab * SW + SW],
                start=True, stop=True,
            )

        pend = (slab, sc_ps, e_tiles[g], xkv, moff)
        moff += ECOL

    if do_comp and pend is not None:
        softmax(pend[0], pend[1], pend[2], pend[4])
        pv(pend[0], pend[2], pend[3])

    # ---- finalization ----
    if not do_comp:
        dummy = fpool.tile([BH, D], F32, tag="dummy")
        nc.vector.memset(dummy[:], 0.0)
        nc.sync.dma_start(out=out[:], in_=dummy[:])
        return
    rnorm = fpool.tile([BH, 1], F32, tag="rnorm")
    nc.vector.reciprocal(out=rnorm[:], in_=acc_ps[:, D:D + 1])
    out_sb = fpool.tile([BH, D], F32, tag="out_sb")
    nc.vector.tensor_scalar_mul(out_sb[:], acc_ps[:, 0:D], rnorm[:])
    nc.sync.dma_start(out=out[:], in_=out_sb[:])


def prepare(inputs):
    """Host-side shard prep. Returns (key, per-core in_maps, perm)."""
    q = np.asarray(inputs["q"], dtype=np.float32)
    k = np.asarray(inputs["k"], dtype=np.float32)
    v = np.asarray(inputs["v"], dtype=np.float32)
    k_cache = np.asarray(inputs["k_cache"], dtype=np.float32)
    v_cache = np.asarray(inputs["v_cache"], dtype=np.float32)
    block_tables = np.asarray(inputs["block_tables"], dtype=np.int32)
    context_lens = np.asarray(inputs["context_lens"], dtype=np.int64)
    slot_mapping = np.asarray(inputs["slot_mapping"], dtype=np.int32)

    perm, nb = plan_batches(context_lens)
    cl_perm = context_lens[perm]  # [16] permuted

    # Dedup within each slab: a block row gathered once can serve several
    # (batch, position) occurrences — scores are computed for all 32 slab
    # columns anyway, so multiplicity/ownership lives entirely in the mask
    # (weight = occurrence count; duplicate blocks in the reference sum
    # twice, so a merged row with weight 2 is numerically identical).
    # Only fully-valid occurrences merge; tail blocks (partial validity)
    # stay as individual rows with per-token masks.
    rows_sl = []
    for slab in range(2):
        merged: dict = {}
        order = []
        tails = []
        for j in range(SLABB):
            b = int(perm[slab * SLABB + j])
            n = int(nb[b])
            ctx = int(context_lens[b])
            for p in range(n):
                blk = int(block_tables[b, p])
                if (p + 1) * BLOCK_SIZE > ctx:   # tail block: partial tokens
                    tails.append((blk, j, p))
                else:
                    if blk not in merged:
                        merged[blk] = {}
                        order.append(blk)
                    merged[blk][j] = merged[blk].get(j, 0) + 1
        rows = [(blk, merged[blk], None) for blk in order]
        rows += [(blk, {j: 1}, (j, p)) for blk, j, p in tails]
        rows_sl.append(rows)

    groups = []
    for slab in range(2):
        tot = len(rows_sl[slab])
        i = 0
        while i < tot:
            c = min(GRP, tot - i)
            groups.append((slab, i, c))
            i += c
    key = tuple((slab, cnt) for slab, _, cnt in groups)
    G = len(groups)

    btt = np.full((GRP, G), OOB, dtype=np.int32)
    msk = np.zeros((GRP, ECOL * G), dtype=np.float32)
    for gi, (slab, i0, cnt) in enumerate(groups):
        moff = ECOL * gi
        for r in range(cnt):
            blk, owners, tail = rows_sl[slab][i0 + r]
            btt[r, gi] = blk
            if tail is None:
                for j, c in owners.items():
                    base = moff + j * N_REP
                    for t in range(BLOCK_SIZE):
                        msk[r, base + t * SW: base + t * SW + N_REP] = c
            else:
                (j, p) = tail
                ctx = int(cl_perm[slab * SLABB + j])
                base = moff + j * N_REP
                for t in range(BLOCK_SIZE):
                    if p * BLOCK_SIZE + t < ctx:
                        msk[r, base + t * SW: base + t * SW + N_REP] = 1.0
    msk = msk.astype(ml_dtypes.float8_e3m4)
    in_maps = []
    for h in range(KV):
        k_sh = np.ascontiguousarray(k_cache[:, :, h, :]).reshape(-1, D)
        v_sh = np.ascontiguousarray(v_cache[:, :, h, :]).reshape(-1, D)
        k_sh[slot_mapping] = k[:, h, :]
        v_sh[slot_mapping] = v[:, h, :]
        kv = np.zeros((NUM_BLOCKS, ROW), dtype=np.uint8)
        k8 = np.clip(k_sh, -15.0, 15.0).astype(ml_dtypes.float8_e3m4)
        kv[:, :KROW] = k8.view(np.uint8).reshape(NUM_BLOCKS, KROW)
        v_ext = np.zeros((NUM_BLOCKS * BLOCK_SIZE, VTOK), dtype=ml_dtypes.bfloat16)
        v_ext[:, :D] = v_sh
        v_ext[:, D] = 1.0
        kv[:, KROW:] = v_ext.view(np.uint8).reshape(NUM_BLOCKS, VROW)
        # qt columns in permuted batch-major order: col (slabpos)*4 + h
        qp = q.reshape(B, KV, N_REP, D)[perm, h, :, :]   # [16, 4, D] permuted
        qtc = np.ascontiguousarray(qp.transpose(2, 0, 1).reshape(D, BH))
        in_maps.append({
            "kvsh": kv,
            "qt": qtc.astype(ml_dtypes.bfloat16),
            "btt": btt,
            "msk": msk,
        })
    return key, in_maps, perm


def build_program(key: tuple, rep: int = 1):
    return build_program_v(key, rep=rep, variant="full")


def kernel(q, k, v, k_cache, v_cache, block_tables, context_lens, slot_mapping):
    key, in_maps, perm = prepare(dict(
        q=q, k=k, v=v, k_cache=k_cache, v_cache=v_cache,
        block_tables=block_tables, context_lens=context_lens,
        slot_mapping=slot_mapping))

    if key not in _kernel_cache:
        _kernel_cache[key] = build_program(key)
    nc = _kernel_cache[key]

    res = run_bass_kernel_spmd(nc, in_maps, core_ids=list(range(KV)))

    inv = np.argsort(perm)
    full = np.empty((B, NUM_HEADS, D), dtype=np.float32)
    for h in range(KV):
        o = res.results[h]["out"].reshape(B, N_REP, D)[inv]
        full[:, h * N_REP:(h + 1) * N_REP, :] = o
    return full
